# revision 1
# baseline (speedup 1.0000x reference)
"""Trainium2 Bass kernel for nn_CausalMultiresConv1d.

Reference computation (per batch b, channel c):
    r_0 = x
    y   = sum_{lvl=0..7} w[:, 8-lvl] * (h1 *_{d=2^lvl} r_lvl)
          + w[:,0] * r_8 + w[:,9] * x,   r_{lvl+1} = h0 *_d r_lvl
    out = gelu(y)   (exact erf gelu; causal depthwise convs, K=4 taps)

Sharding: pure data parallel — 1 batch element per NeuronCore (B=8, 8 cores).
Per-core layout: the [64ch, 32768] slice is packed as [128 partitions, 768+16384]:
  partition p = 64*j + c  ->  channel c, L-half j.
  Leading 768 cols are the causal halo: zeros for half 0, real x for half 1
  (768 >= 765 = total receptive field of the level stack), so both halves
  compute exactly with no inter-chunk communication.

Engine split (default variant hr72 — every h0 level is column-split):
  - TensorE: left ~72% of each h0-conv level as diagonal-matmul taps
    (exact fp32, 4 cyc/col) accumulating 4 shifted rhs views in PSUM.
  - ScalarE: PSUM->SBUF drains, tap-0 scaled copies of the right part,
    y init (w9*x), final exact GELU.
  - VectorE: the 32 h1 y-taps + w0-folded final taps + right-part h0 taps
    1-3 as fused scalar_tensor_tensor multiply-accumulates, with
    per-channel scalars (w (x) h1 folded on the host).
  Measured engine rates (this HW): DVE STT ~14.5us / 16K-wide pass,
  ACT pass ~13.7us, PE fp32 matmul 4 cyc/col. The 36 DVE y-chain passes
  are the capacity floor; the split ratio balances DVE vs PE.
"""

import numpy as np

import concourse.bass as bass
import concourse.mybir as mybir
from concourse.bass_utils import run_bass_kernel_spmd
from concourse.tile import TileContext
from concourse.vector_clock import ScopedClock

# The walrus build here rejects instructions carrying more than one sync-wait
# ("Too many sync wait commands"). Tile's kernel-tail drain attaches a wait for
# every outstanding semaphore to a single SP Drain. _TC splits them: hoist all
# but the last wait onto dedicated single-wait NOPs preceding the drain.


class _TC(TileContext):
    def __exit__(self, *a):
        r = super().__exit__(*a)
        _split_multi_waits(self.nc)
        return r


def _split_multi_waits(nc):
    """Post-pass: for any instruction with >1 sync waits, hoist all but the
    last onto fresh single-wait NOPs on the same engine placed just before
    it (engines execute their stream in order, so semantics are identical)."""
    n = 0
    for fn in nc.m.functions:
        for blk in fn.blocks:
            insts = getattr(blk, "instructions", None)
            if insts is None:
                continue
            new = []
            for inst in insts:
                si = getattr(inst, "sync_info", None)
                waits = list(si.on_wait) if si is not None and si.on_wait else []
                if len(waits) > 1:
                    for j, wcmd in enumerate(waits[:-1]):
                        nop = mybir.InstNoOp(
                            name=f"{inst.name}-hw{j}", engine=inst.engine
                        )
                        nop.sync_info = mybir.SyncInfo(
                            on_wait=[wcmd], on_update=[]
                        )
                        new.append(nop)
                        n += 1
                    inst.sync_info = mybir.SyncInfo(
                        on_wait=[waits[-1]], on_update=list(si.on_update)
                    )
                new.append(inst)
            blk.instructions[:] = new
    return n

B, C, L = 8, 64, 32768
K, DEPTH = 4, 8
NCORES = 8
NCHUNK = 2
CL = L // NCHUNK          # 16384 columns per chunk
PAD = 768                 # halo >= 765 = total receptive field
W = PAD + CL              # 17152 buffer columns
P = NCHUNK * C            # 128 partitions
NS = 41                   # scalar table columns
NDIAG = K * P             # 4 [128,128] diagonal weight matrices
XCOLS = W + NS + NDIAG    # total packed input columns
MMN = 512                 # matmul free-dim tile (one PSUM bank of fp32)

F32 = mybir.dt.float32
MULT = mybir.AluOpType.mult
ADD = mybir.AluOpType.add


def _build_nc(reps=1, variant="hr72"):
    """Build the per-core program. reps>1 repeats the compute phase for
    delta-based wall-clock timing (output is numerically meaningless then,
    because the conv chain scribbles over the input buffer in place)."""
    nc = bass.Bass()
    x_in = nc.dram_tensor("x", [P, XCOLS], F32, kind="ExternalInput")
    y_out = nc.dram_tensor("y", [P, CL], F32, kind="ExternalOutput")

    with _TC(nc) as tc:
        with (
            tc.tile_pool(name="main", bufs=1) as pool,
            tc.tile_pool(name="psum", bufs=6, space="PSUM") as psum_pool,
        ):
            xin = pool.tile([P, XCOLS], F32, tag="xin")
            nxt0 = pool.tile([P, W], F32, tag="nxt")
            y = pool.tile([P, CL], F32, tag="y")

            nc.sync.dma_start(out=xin[:], in_=x_in[:])
            sc = xin[:, W:W + NS]
            diag = [
                xin[:, W + NS + k * P: W + NS + (k + 1) * P] for k in range(K)
            ]

            # which h0-conv levels run on the TensorEngine (exact fp32
            # matmul, 4 cyc/col) vs ACT-tap0 + DVE-taps1..3
            hsplit = None
            if variant == "v1":
                pe_levels = set()
                act_tap0 = False
            elif variant == "v2":
                pe_levels = set(range(DEPTH - 1))
                act_tap0 = False
            elif variant.startswith("g"):
                # "g<N>": N trailing h0 levels on PE, tap0-on-ACT elsewhere
                g = int(variant[1:])
                pe_levels = set(range(DEPTH - 1 - g, DEPTH - 1))
                act_tap0 = True
            else:
                # "hr<P>": every h0 level column-split — left P% of columns
                # on PE, right part on ACT(tap0)+DVE(taps 1-3)
                hsplit = int(variant[2:]) / 100.0
                pe_levels = set()
                act_tap0 = True
            YSEG = 2  # y-pass segmentation for cross-engine pipelining

            for _rep in range(reps):
                cur = xin[:, :W]
                nxt = nxt0

                # y = w9 * x  (ACT scaled copy; Copy allows per-partition scale)
                nc.scalar.activation(
                    out=y[:], in_=cur[:, PAD:],
                    func=mybir.ActivationFunctionType.Copy,
                    scale=sc[:, 40:41],
                )

                def pe_conv(cur, nxt, d, lo, hi):
                    # h0-conv on PE over nxt columns [lo, hi)
                    o0 = lo
                    while o0 < hi:
                        nn = min(MMN, hi - o0)
                        ps = psum_pool.tile([P, MMN], F32, tag="ps")
                        for k in range(K):
                            nc.tensor.matmul(
                                ps[:, :nn],
                                lhsT=diag[k],
                                rhs=cur[:, o0 - k * d: o0 - k * d + nn],
                                start=(k == 0), stop=(k == K - 1),
                            )
                        nc.scalar.copy(out=nxt[:, o0:o0 + nn], in_=ps[:, :nn])
                        o0 += nn

                def dve_conv(cur, nxt, d, lo, hi, use_act):
                    # h0-conv via tap0 scaled-copy + 3 fused MACs, cols [lo,hi)
                    if use_act:
                        nc.scalar.activation(
                            out=nxt[:, lo:hi], in_=cur[:, lo:hi],
                            func=mybir.ActivationFunctionType.Copy,
                            scale=sc[:, 32:33],
                        )
                    else:
                        nc.vector.tensor_scalar(
                            out=nxt[:, lo:hi], in0=cur[:, lo:hi],
                            scalar1=sc[:, 32:33], scalar2=None, op0=MULT,
                        )
                    for k in (1, 2, 3):
                        nc.vector.scalar_tensor_tensor(
                            out=nxt[:, lo:hi],
                            in0=cur[:, lo - k * d: hi - k * d],
                            scalar=sc[:, 32 + k:33 + k],
                            in1=nxt[:, lo:hi],
                            op0=MULT, op1=ADD,
                        )

                def y_taps(cur, d, cols, seg=YSEG):
                    # y[:, s] += sum_k sc[col+k] * cur[:, s+PAD-k*d],
                    # segmented so downstream consumers can start early.
                    bounds = [CL * i // seg for i in range(seg + 1)]
                    for s0, s1 in zip(bounds, bounds[1:]):
                        for k in range(K):
                            nc.vector.scalar_tensor_tensor(
                                out=y[:, s0:s1],
                                in0=cur[:, PAD + s0 - k * d: PAD + s1 - k * d],
                                scalar=sc[:, cols + k:cols + k + 1],
                                in1=y[:, s0:s1],
                                op0=MULT, op1=ADD,
                            )

                V = 0  # first valid column of cur at this level
                d = 1
                for lvl in range(DEPTH):
                    last = lvl == DEPTH - 1
                    if not last:
                        # nxt = h0-conv(cur), valid from column V + 3*d.
                        # Emitted before the y-taps: the chain is the
                        # critical path, y-taps fill engine gaps.
                        start = V + 3 * d
                        if hsplit is not None:
                            mid = start + int((W - start) * hsplit)
                            mid = min(start + ((mid - start + MMN - 1) // MMN)
                                      * MMN, W)
                            pe_conv(cur, nxt, d, start, mid)
                            if mid < W:
                                dve_conv(cur, nxt, d, mid, W, act_tap0)
                        elif lvl in pe_levels:
                            pe_conv(cur, nxt, d, start, W)
                        else:
                            dve_conv(cur, nxt, d, start, W, act_tap0)
                        y_taps(cur, d, lvl * 4)
                        cur, nxt = nxt, cur
                        V = start
                    else:
                        y_taps(cur, d, lvl * 4)
                        # fold w[:,0] into the last h0 conv, accumulate into y
                        y_taps(cur, d, 36)
                    d *= 2

                # segmented gelu so the output DMA streams out as soon as
                # each segment is final
                gseg = [CL * i // 4 for i in range(5)]
                for a, b in zip(gseg, gseg[1:]):
                    nc.scalar.activation(
                        out=y[:, a:b], in_=y[:, a:b],
                        func=mybir.ActivationFunctionType.Gelu,
                    )
                    if _rep == reps - 1:
                        nc.sync.dma_start(out=y_out[:, a:b], in_=y[:, a:b])
    return nc


_NC_CACHE = {}


def _get_nc(reps=1, variant="hr72"):
    key = (reps, variant)
    if key not in _NC_CACHE:
        _NC_CACHE[key] = _build_nc(reps, variant)
    return _NC_CACHE[key]


def _scalar_table(h0, h1, w):
    """[P, NS] per-partition scalar table; partition p holds channel p % 64."""
    # lax.conv is correlation: out[l] = sum_k h[k] * x[l + (k - (K-1))*d],
    # so the tap at shift -k*d carries weight h[K-1-k].
    t = np.zeros((C, NS), np.float32)
    for lvl in range(DEPTH):
        i = DEPTH - lvl
        for k in range(K):
            t[:, lvl * 4 + k] = w[:, i] * h1[:, 0, K - 1 - k]
    for k in range(K):
        t[:, 32 + k] = h0[:, 0, K - 1 - k]
        t[:, 36 + k] = w[:, 0] * h0[:, 0, K - 1 - k]
    t[:, 40] = w[:, DEPTH + 1]
    return np.tile(t, (NCHUNK, 1))


def pack_inputs(x, h0, h1, w):
    """Host-side packing into per-core [P, XCOLS] buffers."""
    sc = _scalar_table(h0, h1, w)
    diag = np.zeros((P, NDIAG), np.float32)
    for k in range(K):
        v = np.tile(h0[:, 0, K - 1 - k], NCHUNK)
        diag[np.arange(P), k * P + np.arange(P)] = v
    in_maps = []
    for b in range(NCORES):
        buf = np.zeros((P, XCOLS), np.float32)
        for j in range(NCHUNK):
            lo = j * CL
            if lo >= PAD:
                buf[j * C:(j + 1) * C, :W] = x[b, :, lo - PAD:lo + CL]
            else:
                buf[j * C:(j + 1) * C, PAD:W] = x[b, :, lo:lo + CL]
        buf[:, W:W + NS] = sc
        buf[:, W + NS:] = diag
        in_maps.append({"x": buf})
    return in_maps


def unpack_outputs(results):
    out = np.empty((B, C, L), np.float32)
    for b, r in enumerate(results):
        yv = r["y"]
        for j in range(NCHUNK):
            out[b, :, j * CL:(j + 1) * CL] = yv[j * C:(j + 1) * C]
    return out


def kernel(x, h0, h1, w, _trace=False, _variant="hr72"):
    import os
    # the axon NTFF trace hook is unavailable here; make sure a stray
    # BASS_TRACE in the environment can't break execution
    os.environ.setdefault("BASS_NEVER_TRACE", "1")

    x = np.asarray(x, np.float32)
    h0 = np.asarray(h0, np.float32)
    h1 = np.asarray(h1, np.float32)
    w = np.asarray(w, np.float32)

    in_maps = pack_inputs(x, h0, h1, w)
    nc = _get_nc(1, _variant)
    try:
        res = run_bass_kernel_spmd(
            nc, in_maps, core_ids=list(range(NCORES)), trace=_trace,
        )
    except Exception:
        # transient "device unrecoverable" failures have been observed on
        # this fleet; one retry usually succeeds
        res = run_bass_kernel_spmd(
            nc, in_maps, core_ids=list(range(NCORES)), trace=_trace,
        )
    out = unpack_outputs(res.results)
    if _trace:
        return out, res
    return out



# revision 7
# speedup vs baseline: 10.5071x; 10.5071x over previous
"""Trainium2 Bass kernel for nn_CausalMultiresConv1d.

Everything before the final GELU is linear: the whole multires stack is
one combined causal FIR filter per channel, F[c, 0:766], computed on the
host as the impulse response of the reference's linear part.

    out[b, c, n] = gelu( sum_tau F[c, tau] * x[b, c, n - tau] )

Sharding: pure data parallel - 1 batch element per NeuronCore (B=8).

Per-core algorithm (transposed layout so the conv is a PE matmul):
  x[c, 16384*h + 128*t + p]  ->  xt[p, 128*(MH + t) + 64*h + c]   (host)
  i.e. positions-within-block on partitions, (block, half, channel) on
  columns, with MH leading halo blocks per half (zeros for half 0, the
  tail of half 0 for half 1) so the causal history is always in-slice.

  For each channel, the FIR becomes M_c banded matmuls accumulated in
  PSUM:   Y_c[p, (t,h)] = sum_m A_m^c.T @ xt[:, block t-m, (h,c)]
  with A_m^c[q, p] = F[c, p - q + 128 m]  (128x128 Toeplitz bands, bf16).
  M_c is per-channel: bands whose tail energy is negligible are dropped
  (total truncation error ~2e-3 relative, tolerance is 2e-2).

  ACT drains PSUM with exact GELU into a transposed bf16 buffer; PE
  transposes each 128-column block back to natural [64h+c, col] layout;
  ACT/DVE/Pool drain those to fp32 and the result DMAs out.
"""

import numpy as np
import ml_dtypes

import concourse.bass as bass
import concourse.mybir as mybir
from concourse.bass_utils import run_bass_kernel_spmd
from concourse.tile import TileContext

# The walrus build here rejects instructions carrying more than one sync-wait
# ("Too many sync wait commands"). Tile's kernel-tail drain attaches a wait for
# every outstanding semaphore to a single SP Drain. _TC splits them: hoist all
# but the last wait onto dedicated single-wait NOPs preceding the drain.


class _TC(TileContext):
    def __exit__(self, *a):
        r = super().__exit__(*a)
        _split_multi_waits(self.nc)
        return r


def _split_multi_waits(nc):
    n = 0
    for fn in nc.m.functions:
        for blk in fn.blocks:
            insts = getattr(blk, "instructions", None)
            if insts is None:
                continue
            new = []
            for inst in insts:
                si = getattr(inst, "sync_info", None)
                waits = list(si.on_wait) if si is not None and si.on_wait else []
                if len(waits) > 1:
                    for j, wcmd in enumerate(waits[:-1]):
                        nop = mybir.InstNoOp(
                            name=f"{inst.name}-hw{j}", engine=inst.engine
                        )
                        nop.sync_info = mybir.SyncInfo(
                            on_wait=[wcmd], on_update=[]
                        )
                        new.append(nop)
                        n += 1
                    inst.sync_info = mybir.SyncInfo(
                        on_wait=[waits[-1]], on_update=list(si.on_update)
                    )
                new.append(inst)
            blk.instructions[:] = new
    return n


B, C, L = 8, 64, 32768
K, DEPTH = 4, 8
NCORES = 8
NH = 2                  # L-halves packed side by side in the channel dim
HL = L // NH            # 16384 positions per half
NB = HL // 128          # 128 blocks of 128 positions per half
P = 128
FLEN = 766              # combined filter support
MAXM = 7                # max 128-tap bands (covers 766 taps)
TRUNC_THR = 1e-6        # per-channel tail energy cutoff (frac of total)
TSEG = 32               # output blocks per DMA segment

F32 = mybir.dt.float32
BF16 = mybir.dt.bfloat16


def _combined_filter(h0, h1, w):
    """Impulse response [C, FLEN] of the linear part, in float64."""
    h0d = h0[:, 0, :].astype(np.float64)
    h1d = h1[:, 0, :].astype(np.float64)
    wd = w.astype(np.float64)

    def dconv(r, h, d):
        out = np.zeros_like(r)
        for k in range(K):
            s = (K - 1 - k) * d
            out[:, s:] += h[:, k:k + 1] * r[:, :FLEN - s]
        return out

    r = np.zeros((C, FLEN))
    r[:, 0] = 1.0
    y = np.zeros((C, FLEN))
    d = 1
    for i in range(DEPTH, 0, -1):
        y += wd[:, i][:, None] * dconv(r, h1d, d)
        r = dconv(r, h0d, d)
        d *= 2
    y += wd[:, 0][:, None] * r
    y[:, 0] += wd[:, -1]
    return y


def _choose_mc(F):
    """Per-channel band count. With nb bands, the worst-covered output
    position (po=0 in a block) only sees taps <= 128*(nb-1), so pick the
    smallest nb whose worst-case dropped tail is negligible."""
    E = F * F
    tot = E.sum()
    mc = []
    for c in range(C):
        nb = MAXM
        for M in range(1, MAXM):
            if E[c, 128 * M:].sum() <= TRUNC_THR * tot:
                nb = M + 1
                break
        mc.append(nb)
    return tuple(mc)


def _build_nc(mc, reps=1):
    nc = bass.Bass()
    mh = max(mc) - 1                      # halo blocks
    xt_cols = (mh + NB) * 128
    na = sum(mc) + 1                      # band matrices + identity
    xt_in = nc.dram_tensor("xt", [P, xt_cols], BF16, kind="ExternalInput")
    am_in = nc.dram_tensor("am", [P, na * 128], BF16, kind="ExternalInput")
    y_out = nc.dram_tensor("y", [P, HL], F32, kind="ExternalOutput")

    GELU = mybir.ActivationFunctionType.Gelu

    with _TC(nc) as tc:
        with (
            tc.tile_pool(name="main", bufs=1) as pool,
            tc.tile_pool(name="yseg", bufs=2) as ypool,
            tc.tile_pool(name="psum", bufs=1, space="PSUM") as psum_pool,
        ):
            xts = pool.tile([P, xt_cols], BF16, tag="xts")
            ams = pool.tile([P, na * 128], BF16, tag="ams")
            tty = pool.tile([P, NB * 128], BF16, tag="tty")

            nc.sync.dma_start(out=xts[:], in_=xt_in[:])
            nc.sync.dma_start(out=ams[:], in_=am_in[:])

            xts3 = xts.rearrange("p (b q) -> p b q", q=128)
            tty3 = tty.rearrange("p (t q) -> p t q", q=128)
            ident = ams[:, (na - 1) * 128: na * 128]

            for _rep in range(reps):
                # conv: per channel, mc[c] banded matmuls accumulated in PSUM
                off = 0
                for c in range(C):
                    ps = psum_pool.tile([P, 128, 2], F32, tag="ps", bufs=4)
                    for m in range(mc[c]):
                        nc.tensor.matmul(
                            ps[:],
                            lhsT=ams[:, (off + m) * 128: (off + m + 1) * 128],
                            rhs=xts3[:, mh - m: mh - m + NB, c::64],
                            start=(m == 0),
                            stop=(m == mc[c] - 1),
                        )
                    off += mc[c]
                    # exact GELU while draining PSUM -> transposed bf16 buffer
                    nc.scalar.activation(
                        out=tty3[:, :, c::64], in_=ps[:], func=GELU,
                    )

                # back-transpose each 128-col block to natural layout + drain
                for s in range(NB // TSEG):
                    yseg = ypool.tile([P, TSEG * 128], F32, tag="yseg")
                    for i in range(TSEG):
                        t = s * TSEG + i
                        psb = psum_pool.tile([P, 128], BF16, tag="psb", bufs=4)
                        nc.tensor.transpose(
                            psb[:], tty[:, t * 128: (t + 1) * 128], ident,
                        )
                        dst = yseg[:, i * 128: (i + 1) * 128]
                        if t % 2 == 1:
                            nc.scalar.copy(out=dst, in_=psb[:])
                        else:
                            nc.vector.tensor_copy(dst, psb[:])
                    if _rep == reps - 1:
                        nc.sync.dma_start(
                            out=y_out[:, s * TSEG * 128: (s + 1) * TSEG * 128],
                            in_=yseg[:],
                        )
    return nc


_NC_CACHE = {}


def _get_nc(mc, reps=1):
    key = (mc, reps)
    if key not in _NC_CACHE:
        _NC_CACHE[key] = _build_nc(mc, reps)
    return _NC_CACHE[key]


def _band_matrices(F, mc):
    """[P, (sum(mc)+1)*128] bf16: per-channel Toeplitz bands + identity."""
    na = sum(mc) + 1
    am = np.zeros((P, na * 128), np.float32)
    q = np.arange(128)
    off = 0
    for c in range(C):
        Fz = np.zeros(127 + 128 * MAXM + 128)
        Fz[127: 127 + FLEN] = F[c]
        win = np.lib.stride_tricks.sliding_window_view(Fz, 128)
        for m in range(mc[c]):
            # A[q, p] = F[c, p - q + 128 m]
            am[:, (off + m) * 128: (off + m + 1) * 128] = win[127 + 128 * m - q]
        off += mc[c]
    am[:, (na - 1) * 128: na * 128] = np.eye(128, dtype=np.float32)
    return am.astype(ml_dtypes.bfloat16)


def pack_inputs(x, h0, h1, w):
    F = _combined_filter(h0, h1, w)
    mc = _choose_mc(F)
    mh = max(mc) - 1
    am = _band_matrices(F, mc)

    in_maps = []
    for bi in range(NCORES):
        xr = np.ascontiguousarray(x[bi]).reshape(C, NH, NB, 128)
        body = xr.transpose(3, 2, 1, 0).reshape(P, NB * 128)
        halo = np.zeros((P, mh, NH, C), np.float32)
        # half 1's causal history is half 0's last mh blocks
        halo[:, :, 1, :] = xr[:, 0, NB - mh:, :].transpose(2, 1, 0)
        xt = np.concatenate(
            [halo.reshape(P, mh * 128), body], axis=1
        ).astype(ml_dtypes.bfloat16)
        in_maps.append({"xt": xt, "am": am})
    return in_maps, mc


def unpack_outputs(results):
    out = np.empty((B, C, L), np.float32)
    for bi, r in enumerate(results):
        yv = r["y"]
        for h in range(NH):
            out[bi, :, h * HL: (h + 1) * HL] = yv[h * C: (h + 1) * C]
    return out


def kernel(x, h0, h1, w, _trace=False):
    import os
    os.environ.setdefault("BASS_NEVER_TRACE", "1")

    x = np.asarray(x, np.float32)
    h0 = np.asarray(h0, np.float32)
    h1 = np.asarray(h1, np.float32)
    w = np.asarray(w, np.float32)

    in_maps, mc = pack_inputs(x, h0, h1, w)
    nc = _get_nc(mc, 1)
    try:
        res = run_bass_kernel_spmd(
            nc, in_maps, core_ids=list(range(NCORES)), trace=_trace,
        )
    except Exception:
        # transient "device unrecoverable" failures have been observed on
        # this fleet; one retry usually succeeds
        res = run_bass_kernel_spmd(
            nc, in_maps, core_ids=list(range(NCORES)), trace=_trace,
        )
    out = unpack_outputs(res.results)
    if _trace:
        return out, res
    return out
